# revision 23
# baseline (speedup 1.0000x reference)
"""Trainium2 Bass kernel for nn_Attention_85633057947969.

reference:
    energy = tanh(x @ W^T + b)            [B, S, H]
    scores = softmax(energy, axis=1)      (over S, per channel)
    context = sum_s scores * x            [B, H]

Sharding: data-parallel over batch across 8 cores (B=16 -> 2 per core).

Per-core algorithm (single pass over x, no max needed because tanh
bounds energy in [-1, 1] so exp never overflows):
    For each s-chunk:
      - DMA x chunk (fp32 HBM -> bf16 SBUF, natural [s, h] layout)
      - PE-transpose to x^T tiles [h, s] (needed as matmul moving operand
        and for the pooling product)
      - PE: energy^T[o, s] = sum_k W^T[hk, o].T @ x^T[hk, s] in PSUM
      - ACT: t = tanh(energy + b) (bias via activation bias port)
      - ACT: e = exp(t), accum_out -> D partial (softmax denominator)
      - DVE: tensor_tensor_reduce(e * x^T) -> N partial (numerator)
    Finalize: context[o] = N[o] / D[o]
"""

from contextlib import ExitStack

import numpy as np

import concourse.bass as bass
import concourse.bacc as bacc
import concourse.mybir as mybir
import concourse.tile as tile
from concourse.masks import make_identity

B, S, H = 16, 4096, 1024
NCORES = 8
B_LOC = B // NCORES  # 2

PB = 128             # partition block
HB = H // PB         # 8 h-blocks (contraction)
OB = H // PB         # 8 o-blocks (output channels)
S_CHUNK = 1024
NCH = S // S_CHUNK   # 4 chunks per batch
ST = S_CHUNK // PB   # 8 s-subtiles per chunk
NMM = 512            # moving free dim per matmul
NH = S_CHUNK // NMM  # 2 matmul halves per chunk

F32 = mybir.dt.float32
BF16 = mybir.dt.bfloat16


def _build_kernel(ctx: ExitStack, tc: tile.TileContext, x_ap, w_ap, b_ap, o_ap):
    nc = tc.nc
    Tanh = mybir.ActivationFunctionType.Tanh
    Exp = mybir.ActivationFunctionType.Exp
    MULT = mybir.AluOpType.mult
    ADD = mybir.AluOpType.add

    const_pool = ctx.enter_context(tc.tile_pool(name="const", bufs=1))
    x_pool = ctx.enter_context(tc.tile_pool(name="xpool", bufs=3))
    xt_pool = ctx.enter_context(tc.tile_pool(name="xtpool", bufs=4))
    act_pool = ctx.enter_context(tc.tile_pool(name="actpool", bufs=3))
    stats_pool = ctx.enter_context(tc.tile_pool(name="stats", bufs=2))
    psum_pool = ctx.enter_context(tc.tile_pool(name="psum", bufs=2, space="PSUM"))

    # --- constants / weights prep ---
    # x loads are the only SWDGE (gpsimd) DMAs — they share one FIFO queue,
    # so chunk 0 must head it.  W and b go through HWDGE (sync) as fp32.
    ident32 = const_pool.tile([PB, PB], F32)
    make_identity(nc, ident32[:])

    def load_chunk(bi, c, name=None):
        # s-order within the chunk is permuted (s = s0 + p*ST + t) so each
        # partition reads contiguous runs; everything downstream reduces
        # over s, so any fixed permutation is fine.  4 separate DMAs per
        # chunk so the xbar transposes can start before the whole chunk
        # has landed.
        x_nat = x_pool.tile(
            [PB, ST, H], BF16, tag="xnat", name=name or f"x_nat_{bi}_{c}"
        )
        src = x_ap[bi, c * S_CHUNK:(c + 1) * S_CHUNK, :].rearrange(
            "(p t) h -> p t h", p=PB
        )
        for g in range(4):
            nc.gpsimd.dma_start(
                x_nat[:, 2 * g:2 * g + 2, :], src[:, 2 * g:2 * g + 2, :]
            )
        return x_nat

    prefetch = [load_chunk(0, 0), load_chunk(0, 1)]

    b_sb = const_pool.tile([PB, OB], F32)
    nc.sync.dma_start(b_sb[:], b_ap.rearrange("(ob p) -> p ob", p=PB))

    # W natural [o, h] -> fp32 SBUF [p, ob, h] via HWDGE, then W^T on PE
    wT = const_pool.tile([PB, HB, OB, PB], BF16)
    with tc.tile_pool(name="wload", bufs=1) as wload_pool:
        w_nat = wload_pool.tile([PB, OB, H], F32)
        nc.sync.dma_start(w_nat[:], w_ap.rearrange("(ob p) h -> p ob h", p=PB))

        # wT[:, k, ob, :] = W[ob-block, k-block]^T  ([h-in-block, o])
        for ob in range(OB):
            wt_ps = psum_pool.tile([PB, HB * PB], F32, tag="eps", bufs=3)
            for k in range(HB):
                nc.tensor.transpose(
                    wt_ps[:, k * PB:(k + 1) * PB],
                    w_nat[:, ob, k * PB:(k + 1) * PB],
                    ident32[:],
                )
            nc.vector.tensor_copy(wT[:, :, ob, :], wt_ps[:])

    ctx_all = const_pool.tile([PB, B_LOC, OB], F32)

    # --- main streaming loop ---
    for bi in range(B_LOC):
        Dparts = stats_pool.tile([PB, OB, NCH], F32, tag="dparts")
        Nparts = stats_pool.tile([PB, OB, NCH], F32, tag="nparts")

        for c in range(NCH):
            if bi == 0 and c < 2:
                x_nat = prefetch[c]
            else:
                x_nat = load_chunk(bi, c)

            # transpose to x^T: [128, HB, S_CHUNK] bf16, partition = h in block
            # (DMA xbar transpose: [128 s, 1024 h] -> logical [1024 h, 128 s],
            #  h = k*128 + p maps onto out dims [p, k, s])
            x_T = xt_pool.tile([PB, HB, S_CHUNK], BF16, tag="xt")
            for t in range(ST):
                nc.sync.dma_start_transpose(
                    x_T[:, :, t * PB:(t + 1) * PB], x_nat[:, t, :]
                )

            for ob in range(OB):
                e_ps = psum_pool.tile([PB, S_CHUNK], F32, tag="eps", bufs=3)
                for h2 in range(NH):
                    for k in range(HB):
                        nc.tensor.matmul(
                            e_ps[:, h2 * NMM:(h2 + 1) * NMM],
                            wT[:, k, ob, :],
                            x_T[:, k, h2 * NMM:(h2 + 1) * NMM],
                            start=(k == 0),
                            stop=(k == HB - 1),
                        )
                t_sb = act_pool.tile([PB, S_CHUNK], F32, tag="tsb")
                nc.scalar.activation(
                    t_sb[:], e_ps[:], Tanh, bias=b_sb[:, ob:ob + 1], scale=1.0
                )
                et = act_pool.tile([PB, S_CHUNK], BF16, tag="et")
                nc.scalar.activation(
                    et[:], t_sb[:], Exp, accum_out=Dparts[:, ob, c:c + 1]
                )
                prod = act_pool.tile([PB, S_CHUNK], BF16, tag="prod")
                nc.vector.tensor_mul(prod[:], et[:], x_T[:, ob, :])
                nc.vector.tensor_reduce(
                    Nparts[:, ob, c:c + 1], prod[:], axis=mybir.AxisListType.X, op=ADD
                )

        # finalize batch: context = N / D per o-block
        for ob in range(OB):
            Ds = stats_pool.tile([PB, 1], F32, tag="fin_d")
            nc.vector.tensor_reduce(
                Ds[:], Dparts[:, ob, :], axis=mybir.AxisListType.X, op=ADD
            )
            Ns = stats_pool.tile([PB, 1], F32, tag="fin_n")
            nc.vector.tensor_reduce(
                Ns[:], Nparts[:, ob, :], axis=mybir.AxisListType.X, op=ADD
            )
            rec = stats_pool.tile([PB, 1], F32, tag="fin_r")
            nc.vector.reciprocal(rec[:], Ds[:])
            nc.vector.tensor_tensor(
                out=ctx_all[:, bi, ob:ob + 1], in0=Ns[:], in1=rec[:], op=MULT
            )

    nc.sync.dma_start(o_ap.rearrange("b (ob p) -> p b ob", p=PB), ctx_all[:])


def build_program():
    nc = bacc.Bacc("TRN2", debug=False)
    x_d = nc.dram_tensor("x", (B_LOC, S, H), F32, kind="ExternalInput")
    w_d = nc.dram_tensor("W", (H, H), F32, kind="ExternalInput")
    b_d = nc.dram_tensor("b", (H,), F32, kind="ExternalInput")
    o_d = nc.dram_tensor("out", (B_LOC, H), F32, kind="ExternalOutput")

    with tile.TileContext(nc) as tc:
        with ExitStack() as ctx:
            _build_kernel(ctx, tc, x_d.ap(), w_d.ap(), b_d.ap(), o_d.ap())
    nc.compile()
    return nc


_cached_nc = None


def _get_nc():
    global _cached_nc
    if _cached_nc is None:
        _cached_nc = build_program()
    return _cached_nc


def kernel(lstm_output, W, b):
    from concourse import bass_utils

    x = np.ascontiguousarray(np.asarray(lstm_output, dtype=np.float32))
    Wn = np.ascontiguousarray(np.asarray(W, dtype=np.float32))
    bn = np.ascontiguousarray(np.asarray(b, dtype=np.float32))

    nc = _get_nc()
    in_maps = [
        {"x": x[i * B_LOC:(i + 1) * B_LOC], "W": Wn, "b": bn}
        for i in range(NCORES)
    ]
    res = bass_utils.run_bass_kernel_spmd(nc, in_maps, core_ids=list(range(NCORES)))
    out = np.concatenate([res.results[i]["out"] for i in range(NCORES)], axis=0)
    return out.astype(np.float32)


# revision 25
# speedup vs baseline: 1.1987x; 1.1987x over previous
"""Trainium2 Bass kernel for nn_Attention_85633057947969.

reference:
    energy = tanh(x @ W^T + b)            [B, S, H]
    scores = softmax(energy, axis=1)      (over S, per channel)
    context = sum_s scores * x            [B, H]

Sharding: data-parallel over batch across 8 cores (B=16 -> 2 per core).

Per-core algorithm (single pass over x, no max needed because tanh
bounds energy in [-1, 1] so exp never overflows):
    For each s-chunk:
      - DMA x chunk (fp32 HBM -> bf16 SBUF, natural [s, h] layout)
      - PE-transpose to x^T tiles [h, s] (needed as matmul moving operand
        and for the pooling product)
      - PE: energy^T[o, s] = sum_k W^T[hk, o].T @ x^T[hk, s] in PSUM
      - ACT: t = tanh(energy + b) (bias via activation bias port)
      - ACT: e = exp(t), accum_out -> D partial (softmax denominator)
      - DVE: tensor_tensor_reduce(e * x^T) -> N partial (numerator)
    Finalize: context[o] = N[o] / D[o]
"""

from contextlib import ExitStack

import numpy as np

import concourse.bass as bass
import concourse.bacc as bacc
import concourse.mybir as mybir
import concourse.tile as tile
from concourse.masks import make_identity

B, S, H = 16, 4096, 1024
NCORES = 8
B_LOC = B // NCORES  # 2

PB = 128             # partition block
HB = H // PB         # 8 h-blocks (contraction)
OB = H // PB         # 8 o-blocks (output channels)
S_CHUNK = 1024
NCH = S // S_CHUNK   # 4 chunks per batch
ST = S_CHUNK // PB   # 8 s-subtiles per chunk
NMM = 512            # moving free dim per matmul
NH = S_CHUNK // NMM  # 2 matmul halves per chunk

F32 = mybir.dt.float32
BF16 = mybir.dt.bfloat16


def _build_kernel(ctx: ExitStack, tc: tile.TileContext, x_ap, w_ap, b_ap, o_ap):
    nc = tc.nc
    Tanh = mybir.ActivationFunctionType.Tanh
    Exp = mybir.ActivationFunctionType.Exp
    MULT = mybir.AluOpType.mult
    ADD = mybir.AluOpType.add

    const_pool = ctx.enter_context(tc.tile_pool(name="const", bufs=1))
    x_pool = ctx.enter_context(tc.tile_pool(name="xpool", bufs=3))
    xt_pool = ctx.enter_context(tc.tile_pool(name="xtpool", bufs=3))
    act_pool = ctx.enter_context(tc.tile_pool(name="actpool", bufs=3))
    stats_pool = ctx.enter_context(tc.tile_pool(name="stats", bufs=2))
    psum_pool = ctx.enter_context(tc.tile_pool(name="psum", bufs=2, space="PSUM"))

    # --- constants / weights prep ---
    # x loads are the only SWDGE (gpsimd) DMAs — they share one FIFO queue,
    # so chunk 0 must head it.  W and b go through HWDGE (sync) as fp32.
    ident32 = const_pool.tile([PB, PB], F32)
    make_identity(nc, ident32[:])

    prefetch = []
    for pi in range(2):
        x_nat = x_pool.tile([PB, ST, H], BF16, tag="xnat", name=f"x_nat_p{pi}")
        nc.gpsimd.dma_start(
            x_nat[:],
            x_ap[0, pi * S_CHUNK:(pi + 1) * S_CHUNK, :].rearrange(
                "(p t) h -> p t h", p=PB
            ),
        )
        prefetch.append(x_nat)

    b_sb = const_pool.tile([PB, OB], F32)
    nc.sync.dma_start(b_sb[:], b_ap.rearrange("(ob p) -> p ob", p=PB))

    # W natural [o, h] -> fp32 SBUF [p, ob, h] via HWDGE, then W^T on PE
    wT = const_pool.tile([PB, HB, OB, PB], BF16)
    with tc.tile_pool(name="wload", bufs=1) as wload_pool:
        w_nat = wload_pool.tile([PB, OB, H], F32)
        nc.sync.dma_start(w_nat[:], w_ap.rearrange("(ob p) h -> p ob h", p=PB))

        # wT[:, k, ob, :] = W[ob-block, k-block]^T  ([h-in-block, o])
        for ob in range(OB):
            wt_ps = psum_pool.tile([PB, HB * PB], F32, tag="eps", bufs=3)
            for k in range(HB):
                nc.tensor.transpose(
                    wt_ps[:, k * PB:(k + 1) * PB],
                    w_nat[:, ob, k * PB:(k + 1) * PB],
                    ident32[:],
                )
            nc.vector.tensor_copy(wT[:, :, ob, :], wt_ps[:])

    ctx_all = const_pool.tile([PB, B_LOC, OB], F32)

    # --- main streaming loop ---
    for bi in range(B_LOC):
        Dparts = stats_pool.tile([PB, OB, NCH], F32, tag="dparts")
        Nparts = stats_pool.tile([PB, OB, NCH], F32, tag="nparts")

        for c in range(NCH):
            s0 = c * S_CHUNK
            # load + cast x chunk: [128, ST, H] bf16.
            # s-order within the chunk is permuted (s = s0 + p*ST + t) so each
            # partition reads one 32 KiB contiguous run; everything downstream
            # reduces over s, so any fixed permutation is fine.
            if bi == 0 and c < 2:
                x_nat = prefetch[c]
            else:
                x_nat = x_pool.tile([PB, ST, H], BF16, tag="xnat")
                nc.gpsimd.dma_start(
                    x_nat[:],
                    x_ap[bi, s0:s0 + S_CHUNK, :].rearrange(
                        "(p t) h -> p t h", p=PB
                    ),
                )

            # transpose to x^T: [128, HB, S_CHUNK] bf16, partition = h in block
            # (DMA xbar transpose: [128 s, 1024 h] -> logical [1024 h, 128 s],
            #  h = k*128 + p maps onto out dims [p, k, s])
            x_T = xt_pool.tile([PB, HB, S_CHUNK], BF16, tag="xt")
            for t in range(ST):
                nc.sync.dma_start_transpose(
                    x_T[:, :, t * PB:(t + 1) * PB], x_nat[:, t, :]
                )

            for ob in range(OB):
                e_ps = psum_pool.tile([PB, S_CHUNK], F32, tag="eps", bufs=3)
                for h2 in range(NH):
                    for k in range(HB):
                        nc.tensor.matmul(
                            e_ps[:, h2 * NMM:(h2 + 1) * NMM],
                            wT[:, k, ob, :],
                            x_T[:, k, h2 * NMM:(h2 + 1) * NMM],
                            start=(k == 0),
                            stop=(k == HB - 1),
                        )
                t_sb = act_pool.tile([PB, S_CHUNK], F32, tag="tsb")
                nc.scalar.activation(
                    t_sb[:], e_ps[:], Tanh, bias=b_sb[:, ob:ob + 1], scale=1.0
                )
                et = act_pool.tile([PB, S_CHUNK], BF16, tag="et")
                nc.scalar.activation(
                    et[:], t_sb[:], Exp, accum_out=Dparts[:, ob, c:c + 1]
                )
                prod = act_pool.tile([PB, S_CHUNK], BF16, tag="prod")
                nc.vector.tensor_mul(prod[:], et[:], x_T[:, ob, :])
                nc.vector.tensor_reduce(
                    Nparts[:, ob, c:c + 1], prod[:], axis=mybir.AxisListType.X, op=ADD
                )

        # finalize batch: context = N / D per o-block
        for ob in range(OB):
            Ds = stats_pool.tile([PB, 1], F32, tag="fin_d")
            nc.vector.tensor_reduce(
                Ds[:], Dparts[:, ob, :], axis=mybir.AxisListType.X, op=ADD
            )
            Ns = stats_pool.tile([PB, 1], F32, tag="fin_n")
            nc.vector.tensor_reduce(
                Ns[:], Nparts[:, ob, :], axis=mybir.AxisListType.X, op=ADD
            )
            rec = stats_pool.tile([PB, 1], F32, tag="fin_r")
            nc.vector.reciprocal(rec[:], Ds[:])
            nc.vector.tensor_tensor(
                out=ctx_all[:, bi, ob:ob + 1], in0=Ns[:], in1=rec[:], op=MULT
            )

    # Store: transpose [128, b*ob] -> [b*ob, 128] first so the DMA writes 16
    # contiguous 512B runs instead of a 2048-element 4B scatter.
    fin_ps = psum_pool.tile([B_LOC * OB, PB], F32, tag="eps", bufs=3)
    nc.tensor.transpose(fin_ps[:], ctx_all[:], ident32[:])
    fin_sb = stats_pool.tile([B_LOC * OB, PB], F32, tag="fin_t")
    nc.vector.tensor_copy(fin_sb[:], fin_ps[:])
    nc.sync.dma_start(o_ap.rearrange("b (ob p) -> (b ob) p", p=PB), fin_sb[:])


def build_program():
    nc = bacc.Bacc("TRN2", debug=False)
    x_d = nc.dram_tensor("x", (B_LOC, S, H), F32, kind="ExternalInput")
    w_d = nc.dram_tensor("W", (H, H), F32, kind="ExternalInput")
    b_d = nc.dram_tensor("b", (H,), F32, kind="ExternalInput")
    o_d = nc.dram_tensor("out", (B_LOC, H), F32, kind="ExternalOutput")

    with tile.TileContext(nc) as tc:
        with ExitStack() as ctx:
            _build_kernel(ctx, tc, x_d.ap(), w_d.ap(), b_d.ap(), o_d.ap())
    nc.compile()
    return nc


_cached_nc = None


def _get_nc():
    global _cached_nc
    if _cached_nc is None:
        _cached_nc = build_program()
    return _cached_nc


def kernel(lstm_output, W, b):
    from concourse import bass_utils

    x = np.ascontiguousarray(np.asarray(lstm_output, dtype=np.float32))
    Wn = np.ascontiguousarray(np.asarray(W, dtype=np.float32))
    bn = np.ascontiguousarray(np.asarray(b, dtype=np.float32))

    nc = _get_nc()
    in_maps = [
        {"x": x[i * B_LOC:(i + 1) * B_LOC], "W": Wn, "b": bn}
        for i in range(NCORES)
    ]
    res = bass_utils.run_bass_kernel_spmd(nc, in_maps, core_ids=list(range(NCORES)))
    out = np.concatenate([res.results[i]["out"] for i in range(NCORES)], axis=0)
    return out.astype(np.float32)
